# revision 13
# baseline (speedup 1.0000x reference)
"""Trainium2 Bass kernel for NeuralImplicitAccessibilityField.

Data-parallel over N=262144 query points on 8 NeuronCores (32768 queries
per core); residues, anchors and weights replicated.

Outputs (matching reference): (signed_distance[N], accessibility[N],
steric_loss, read_w[N,128], context[N,64]).
"""
import sys
import numpy as np

sys.path.insert(0, "/opt/trn_rl_repo")

import concourse.bass as bass  # noqa: E402
import concourse.bacc as bacc  # noqa: E402
import concourse.tile as tile  # noqa: E402
from concourse import mybir  # noqa: E402

F32 = mybir.dt.float32
BF16 = mybir.dt.bfloat16
AF = mybir.ActivationFunctionType
ALU = mybir.AluOpType
AX = mybir.AxisListType

N_CORES = 8
M = 1024      # residues
A = 128       # anchors
MEM = 64
SUB = 128     # queries per subtile
GRP = 512     # queries per group (4 subtiles)



# ---------------------------------------------------------------------------

def build_module(nq: int, radius: float, sr_b2: float):
    import os
    PH = os.environ.get("K_PHASES", "ABD")
    """Build the per-core Bass module for nq queries (nq % 2048 == 0)."""
    nsub = nq // SUB          # subtiles of 128 queries
    ngrp = nq // GRP          # groups of 512 queries
    npass = (nsub + 127) // 128  # passes of <=128 subtiles
    sub_per_pass = nsub // npass

    nc = bacc.Bacc("TRN2", target_bir_lowering=False, debug=False,
                   num_devices=N_CORES)

    # ---- DRAM parameters -------------------------------------------------
    din = {}

    def dram_in(name, shape):
        din[name] = nc.declare_dram_parameter(name, list(shape), F32,
                                              isOutput=False)
        return din[name]

    c5t = dram_in("c5t", [5, nq])
    r5 = dram_in("r5", [5, M])
    keys2t = dram_in("keys2t", [MEM, 2 * A])
    cpw1 = dram_in("cpw1", [3, MEM])
    cpb1 = dram_in("cpb1", [MEM, 1])
    cpb1h = dram_in("cpb1h", [MEM, 1])
    cpw2h = dram_in("cpw2h", [MEM, MEM])
    cpb2 = dram_in("cpb2", [MEM, 1])
    valh = dram_in("valh", [A, MEM])
    ctrlh = dram_in("ctrlh", [A, MEM])
    slots = dram_in("slots", [A, MEM])
    srw1 = dram_in("srw1", [2 * MEM, MEM])
    wsdfrep = dram_in("wsdfrep", [128, MEM])
    srb1 = dram_in("srb1", [MEM, 1])
    srb1h = dram_in("srb1h", [MEM, 1])
    srw2h = dram_in("srw2h", [MEM, 1])
    ident = dram_in("ident", [128, 128])

    sd_cols_d = nc.declare_dram_parameter("sd_cols", [128, nsub], F32,
                                          isOutput=True)
    acc_cols_d = nc.declare_dram_parameter("acc_cols", [128, nsub], F32,
                                           isOutput=True)
    readw_d = nc.declare_dram_parameter("readw", [nq, A], F32, isOutput=True)
    ctxt_d = nc.declare_dram_parameter("ctxt", [MEM, nq], F32, isOutput=True)
    steric_d = nc.declare_dram_parameter("steric", [128, 1], F32,
                                         isOutput=True)
    resrow_d = nc.dram_tensor("resrow", [nq // GRP, GRP], F32)

    with tile.TileContext(nc) as tc:
        import contextlib
        with contextlib.ExitStack() as ctx:
            singles = ctx.enter_context(tc.tile_pool(name="singles", bufs=1))
            persist = ctx.enter_context(tc.tile_pool(name="persist", bufs=1))

            # ---- constants into SBUF ------------------------------------
            def load_const(ap, shape):
                nm = ap.name + "_s"
                t = singles.tile(list(shape), F32, name=nm, tag=nm)
                nc.sync.dma_start(t[:], ap[:])
                return t

            r5_s = load_const(r5, [5, M])
            keys2t_s = load_const(keys2t, [MEM, 2 * A])
            cpw1_s = load_const(cpw1, [3, MEM])
            cpb1_s = load_const(cpb1, [MEM, 1])
            cpb1h_s = load_const(cpb1h, [MEM, 1])
            cpw2h_s = load_const(cpw2h, [MEM, MEM])
            cpb2_s = load_const(cpb2, [MEM, 1])
            valh_s = load_const(valh, [A, MEM])
            ctrlh_s = load_const(ctrlh, [A, MEM])
            slots_s = load_const(slots, [A, MEM])
            srw1_s = load_const(srw1, [2 * MEM, MEM])
            wsdfrep_s = load_const(wsdfrep, [128, MEM])
            srb1_s = load_const(srb1, [MEM, 1])
            srb1h_s = load_const(srb1h, [MEM, 1])
            srw2h_s = load_const(srw2h, [MEM, 1])
            ident_s = load_const(ident, [128, 128])

            # ---- persistent SBUF tiles ----------------------------------
            qts_all = persist.tile([MEM, nq], F32)        # q^T, feature-major
            min2_cols = persist.tile([128, nsub], F32)    # min d2 per subtile
            sdf_cols = persist.tile([128, nsub], F32)
            sd_cols = persist.tile([128, nsub], F32)
            acc_cols = persist.tile([128, nsub], F32)

            # =============================================================
            # Phase A: distance field (d2 + min) and q-MLP
            # =============================================================
            if "A" in PH:
              with contextlib.ExitStack() as actx:
                pool_a = actx.enter_context(tc.tile_pool(name="pa", bufs=2))
                psum_d2p = actx.enter_context(
                    tc.tile_pool(name="pd2", bufs=2, space="PSUM"))
                psum_mlp = actx.enter_context(
                    tc.tile_pool(name="pmlp", bufs=2, space="PSUM"))

                for blk in range(nq // 2048):     # 2048-query superchunks
                    cchunk = pool_a.tile([5, 2048], F32, tag="cchunk")
                    nc.sync.dma_start(cchunk[:],
                                      c5t[:, blk * 2048:(blk + 1) * 2048])
                    for gi in range(4):           # groups of 512 in chunk
                        g = blk * 4 + gi
                        # --- cp MLP (q^T) over 512 queries ---------------
                        ph1 = psum_mlp.tile([MEM, GRP], F32, tag="ph1")
                        nc.tensor.matmul(
                            ph1[:], cpw1_s[:],
                            cchunk[0:3, gi * GRP:(gi + 1) * GRP])
                        u1 = pool_a.tile([MEM, GRP], F32, tag="u1")
                        nc.scalar.activation(u1[:], ph1[:], AF.Identity,
                                             bias=cpb1_s[:])
                        t1 = pool_a.tile([MEM, GRP], F32, tag="t1")
                        nc.scalar.activation(t1[:], ph1[:], AF.Tanh,
                                             bias=cpb1h_s[:], scale=0.5)
                        m1 = pool_a.tile([MEM, GRP], F32, tag="m1")
                        nc.vector.tensor_mul(m1[:], u1[:], t1[:])
                        pqt = psum_mlp.tile([MEM, GRP], F32, tag="pqt")
                        nc.tensor.matmul(pqt[:], cpw2h_s[:], u1[:],
                                         start=True, stop=False)
                        nc.tensor.matmul(pqt[:], cpw2h_s[:], m1[:],
                                         start=False, stop=True)
                        nc.scalar.activation(
                            qts_all[:, g * GRP:(g + 1) * GRP], pqt[:],
                            AF.Identity, bias=cpb2_s[:])

                        # --- d2 + min per subtile ------------------------
                        for j in range(4):
                            t = g * 4 + j
                            pd2 = psum_d2p.tile([128, M], F32, tag="pd2")
                            lhs = cchunk[:, gi * GRP + j * SUB:
                                         gi * GRP + (j + 1) * SUB]
                            nc.tensor.matmul(pd2[:, 0:512], lhs, r5_s[:, 0:512])
                            nc.tensor.matmul(pd2[:, 512:1024], lhs,
                                             r5_s[:, 512:1024])
                            nc.vector.tensor_reduce(
                                min2_cols[:, t:t + 1], pd2[:],
                                axis=AX.X, op=ALU.min)

            # =============================================================
            # Interlude: sdf_base = sqrt(max(min2,0)) - radius; transposes
            # =============================================================
            if "A" in PH:
              with contextlib.ExitStack() as ictx:
                ipool = ictx.enter_context(tc.tile_pool(name="ip", bufs=1))
                ipsum = ictx.enter_context(
                    tc.tile_pool(name="ips", bufs=1, space="PSUM"))
                clamped = ipool.tile([128, nsub], F32)
                nc.vector.tensor_scalar_max(clamped[:], min2_cols[:], 0.0)
                dist = ipool.tile([128, nsub], F32)
                nc.scalar.activation(dist[:], clamped[:], AF.Sqrt)
                nc.vector.tensor_scalar_add(sdf_cols[:], dist[:], -radius)

            # =============================================================
            # Phase B: attention + feature build + final MLP per group
            # =============================================================
            if "B" in PH:
              with contextlib.ExitStack() as bctx:
                pool_b = bctx.enter_context(tc.tile_pool(name="pb", bufs=2))
                ps_log = bctx.enter_context(
                    tc.tile_pool(name="pslog", bufs=1, space="PSUM"))
                ps_tr = bctx.enter_context(
                    tc.tile_pool(name="pstr", bufs=1, space="PSUM"))
                ps_feat = bctx.enter_context(
                    tc.tile_pool(name="psfeat", bufs=1, space="PSUM"))
                ps_ht = bctx.enter_context(
                    tc.tile_pool(name="psht", bufs=1, space="PSUM"))
                ps_res = bctx.enter_context(
                    tc.tile_pool(name="psres", bufs=1, space="PSUM"))

                readw_v = readw_d.ap().rearrange("(t p) c -> p t c", p=SUB)

                for g in range(ngrp):
                    p = (g * 4) // sub_per_pass
                    # --- logits ------------------------------------------
                    plog = ps_log.tile([128, 2 * GRP], F32, tag="plog")
                    for j in range(4):
                        t = g * 4 + j
                        qslice = qts_all[:, t * SUB:(t + 1) * SUB]
                        nc.tensor.matmul(plog[:, j * SUB:(j + 1) * SUB],
                                         qslice, keys2t_s[:, 0:A])
                        nc.tensor.matmul(plog[:, GRP + j * SUB:
                                              GRP + (j + 1) * SUB],
                                         qslice, keys2t_s[:, A:2 * A])
                    expa = pool_b.tile([128, GRP], F32, tag="expa")
                    nc.scalar.activation(expa[:], plog[:, 0:GRP], AF.Exp)
                    expr = pool_b.tile([128, GRP], F32, tag="expr")
                    nc.scalar.activation(expr[:], plog[:, GRP:2 * GRP], AF.Exp)

                    sums8 = pool_b.tile([128, 8], F32, tag="sums8")
                    nc.vector.tensor_reduce(
                        sums8[:, 0:4], expa.rearrange("p (j a) -> p j a", j=4),
                        axis=AX.X, op=ALU.add)
                    nc.vector.tensor_reduce(
                        sums8[:, 4:8], expr.rearrange("p (j a) -> p j a", j=4),
                        axis=AX.X, op=ALU.add)
                    recip8 = pool_b.tile([128, 8], F32, tag="recip8")
                    if os.environ.get("K_NORECIP"):
                        nc.vector.tensor_copy(recip8[:], sums8[:])
                    else:
                        nc.vector.reciprocal(recip8[:], sums8[:])

                    attn_n = pool_b.tile([128, GRP], F32, tag="attn_n")
                    readw_n = pool_b.tile([128, GRP], F32, tag="readw_n")
                    for j in range(4):
                        sl = slice(j * SUB, (j + 1) * SUB)
                        nc.vector.tensor_scalar_mul(
                            attn_n[:, sl], expa[:, sl], recip8[:, j:j + 1])
                        nc.vector.tensor_scalar_mul(
                            readw_n[:, sl], expr[:, sl], recip8[:, 4 + j:5 + j])
                    nc.sync.dma_start(
                        readw_v[:, g * 4:(g + 1) * 4, :],
                        readw_n.rearrange("p (j a) -> p j a", j=4))

                    if os.environ.get("K_BSTOP") == "1":
                        continue
                    # --- transposes --------------------------------------
                    ptr = ps_tr.tile([128, 2 * GRP], F32, tag="ptr")
                    for j in range(4):
                        sl = slice(j * SUB, (j + 1) * SUB)
                        nc.tensor.transpose(ptr[:, sl], attn_n[:, sl],
                                            ident_s[:])
                        nc.tensor.transpose(
                            ptr[:, GRP + j * SUB:GRP + (j + 1) * SUB],
                            readw_n[:, sl], ident_s[:])
                    ats = pool_b.tile([128, GRP], F32, tag="ats")
                    nc.scalar.copy(ats[:], ptr[:, 0:GRP])
                    rts = pool_b.tile([128, GRP], F32, tag="rts")
                    nc.vector.tensor_copy(rts[:], ptr[:, GRP:2 * GRP])

                    # --- context / memctx (feature-major) ----------------
                    pfeat = ps_feat.tile([128, GRP], F32, tag="pfeat")
                    for j in range(4):
                        sl = slice(j * SUB, (j + 1) * SUB)
                        nc.tensor.matmul(pfeat[0:MEM, sl], valh_s[:],
                                         ats[:, sl], start=True, stop=False)
                        nc.tensor.matmul(pfeat[0:MEM, sl], ctrlh_s[:],
                                         rts[:, sl], start=False, stop=True)
                        nc.tensor.matmul(pfeat[MEM:2 * MEM, sl], slots_s[:],
                                         rts[:, sl])
                    featt = pool_b.tile([128, GRP], F32, tag="featt")
                    nc.scalar.copy(featt[:], pfeat[:])
                    nc.sync.dma_start(ctxt_d[:, g * GRP:(g + 1) * GRP],
                                      featt[0:MEM, :])

                    if os.environ.get("K_BSTOP") == "2":
                        continue
                    # --- final MLP ---------------------------------------
                    pht = ps_ht.tile([MEM, GRP], F32, tag="pht")
                    nc.tensor.matmul(pht[:], srw1_s[:], featt[:],
                                     start=True, stop=False)
                    for j in range(4):
                        t = g * 4 + j
                        diag_t = pool_b.tile([128, 128], F32, tag="diag_t")
                        nc.vector.tensor_scalar_mul(
                            diag_t[:], ident_s[:], sdf_cols[:, t:t + 1])
                        nc.tensor.matmul(
                            pht[:, j * SUB:(j + 1) * SUB], wsdfrep_s[:],
                            diag_t[:], start=False, stop=(j == 3))
                    u2 = pool_b.tile([MEM, GRP], F32, tag="u2")
                    nc.scalar.activation(u2[:], pht[:], AF.Identity,
                                         bias=srb1_s[:])
                    t2 = pool_b.tile([MEM, GRP], F32, tag="t2")
                    nc.scalar.activation(t2[:], pht[:], AF.Tanh,
                                         bias=srb1h_s[:], scale=0.5)
                    m2 = pool_b.tile([MEM, GRP], F32, tag="m2")
                    nc.vector.tensor_mul(m2[:], u2[:], t2[:])
                    if os.environ.get("K_BSTOP") == "3":
                        continue
                    pres = ps_res.tile([1, GRP], F32, tag="pres")
                    nc.tensor.matmul(pres[:], srw2h_s[:], u2[:],
                                     start=True, stop=False)
                    nc.tensor.matmul(pres[:], srw2h_s[:], m2[:],
                                     start=False, stop=True)
                    if os.environ.get("K_BSTOP") == "4":
                        continue
                    rrow = pool_b.tile([1, GRP], F32, tag="rrow")
                    nc.vector.tensor_copy(rrow[:], pres[:])
                    nc.sync.dma_start(resrow_d[g:g + 1, :], rrow[:])

            # =============================================================
            # Phase D: sd, accessibility, steric, DMAs
            # =============================================================
            if "D" in PH:
              with contextlib.ExitStack() as dctx:
                dpool = dctx.enter_context(tc.tile_pool(name="dp", bufs=1))
                dpsum = dctx.enter_context(
                    tc.tile_pool(name="dps", bufs=1, space="PSUM"))
                rcols_raw = dpool.tile([128, nsub], F32, tag="rcols_raw")
                nc.sync.dma_start(
                    rcols_raw[:],
                    resrow_d.ap().rearrange("g (j p) -> p (g j)", p=SUB))
                rc = dpool.tile([128, nsub], F32, tag="rc")
                nc.scalar.activation(rc[:], rcols_raw[:], AF.Tanh,
                                     bias=float(sr_b2))
                nc.vector.tensor_scalar_mul(rc[:], rc[:], 0.25)
                nc.vector.tensor_add(sd_cols[:], sdf_cols[:], rc[:])
                ta = dpool.tile([128, nsub], F32, tag="ta")
                nc.scalar.activation(ta[:], sd_cols[:], AF.Tanh, scale=4.0)
                nc.vector.tensor_scalar(
                    out=acc_cols[:], in0=ta[:], scalar1=0.5, scalar2=0.5,
                    op0=ALU.mult, op1=ALU.add)
                relu_t = dpool.tile([128, nsub], F32, tag="relu_t")
                steric_t = dpool.tile([128, 1], F32, tag="steric_t")
                nc.scalar.activation(relu_t[:], sd_cols[:], AF.Relu,
                                     scale=-1.0, accum_out=steric_t[:])
                nc.sync.dma_start(sd_cols_d[:], sd_cols[:])
                nc.sync.dma_start(acc_cols_d[:], acc_cols[:])
                nc.sync.dma_start(steric_d[:], steric_t[:])

    if not nc.is_finalized():
        nc.finalize()
    return nc


# ---------------------------------------------------------------------------

def _prep_host(inputs, nq_core):
    """Host-side preprocessing: per-core shards + replicated constants."""
    f32 = np.float32
    coords = np.ascontiguousarray(inputs["coords"], dtype=f32)
    res = np.ascontiguousarray(inputs["residue_positions"], dtype=f32)
    rtypes = np.asarray(inputs["residue_types"]).astype(np.int64)
    radii = np.asarray(inputs["radius_table"], dtype=f32)[rtypes][:, 0]
    uradii = np.unique(radii)
    assert uradii.size == 1, "kernel assumes a single residue radius"
    radius = float(uradii[0])

    n = coords.shape[0]
    c5t = np.empty((5, n), dtype=f32)
    c5t[0:3] = coords.T
    c5t[3] = 1.0
    c5t[4] = (coords.astype(np.float32) ** 2).sum(-1)

    r5 = np.empty((5, M), dtype=f32)
    r5[0:3] = (-2.0 * res).T
    r5[3] = (res ** 2).sum(-1)
    r5[4] = 1.0

    def silu(x):
        return x / (1.0 + np.exp(-x))

    sf = np.asarray(inputs["scalar_features"], dtype=f32)
    aa = np.asarray(inputs["attention_anchors"], dtype=f32)
    mem_in = np.concatenate([sf, aa], -1)
    ctrl = (silu(mem_in @ inputs["mw_w1"] + inputs["mw_b1"])
            @ inputs["mw_w2"] + inputs["mw_b2"]).astype(f32)
    keys = (aa @ inputs["ak_w"] + inputs["ak_b"]).astype(f32)
    values = (mem_in @ inputs["av_w"] + inputs["av_b"]).astype(f32)
    nk = np.asarray(inputs["nftm_keys"], dtype=f32)
    nslots = np.asarray(inputs["nftm_slots"], dtype=f32)

    const = {
        "r5": r5,
        "keys2t": np.concatenate([keys.T, nk.T], axis=1) / 8.0,
        "cpw1": np.asarray(inputs["cp_w1"], dtype=f32),
        "cpb1": np.asarray(inputs["cp_b1"], dtype=f32).reshape(MEM, 1),
        "cpb1h": 0.5 * np.asarray(inputs["cp_b1"], dtype=f32).reshape(MEM, 1),
        "cpw2h": 0.5 * np.asarray(inputs["cp_w2"], dtype=f32),
        "cpb2": np.asarray(inputs["cp_b2"], dtype=f32).reshape(MEM, 1),
        "valh": 0.5 * values,
        "ctrlh": 0.5 * ctrl,
        "slots": nslots,
        "srw1": np.asarray(inputs["sr_w1"], dtype=f32)[0:2 * MEM],
        "wsdfrep": np.tile(
            np.asarray(inputs["sr_w1"], dtype=f32)[2 * MEM:2 * MEM + 1],
            (128, 1)),
        "srb1": np.asarray(inputs["sr_b1"], dtype=f32).reshape(MEM, 1),
        "srb1h": 0.5 * np.asarray(inputs["sr_b1"], dtype=f32).reshape(MEM, 1),
        "srw2h": 0.5 * np.asarray(inputs["sr_w2"], dtype=f32),
        "ident": np.eye(128, dtype=f32),
    }
    const = {k: np.ascontiguousarray(v, dtype=f32) for k, v in const.items()}

    in_maps = []
    for c in range(N_CORES):
        m = dict(const)
        m["c5t"] = np.ascontiguousarray(
            c5t[:, c * nq_core:(c + 1) * nq_core])
        in_maps.append(m)
    sr_b2 = float(np.asarray(inputs["sr_b2"]).reshape(-1)[0])
    return in_maps, radius, sr_b2


def kernel(**inputs):
    from concourse.bass_utils import run_bass_kernel_spmd

    n = inputs["coords"].shape[0]
    assert n % N_CORES == 0
    nq_core = n // N_CORES

    in_maps, radius, sr_b2 = _prep_host(inputs, nq_core)
    nc = build_module(nq_core, radius, sr_b2)
    res = run_bass_kernel_spmd(nc, in_maps, list(range(N_CORES)))

    nsub = nq_core // SUB
    sd = np.empty(n, dtype=np.float32)
    acc = np.empty(n, dtype=np.float32)
    read_w = np.empty((n, A), dtype=np.float32)
    context = np.empty((n, MEM), dtype=np.float32)
    steric_sum = 0.0
    for c in range(N_CORES):
        o = res.results[c]
        sl = slice(c * nq_core, (c + 1) * nq_core)
        sd[sl] = o["sd_cols"].T.reshape(-1)
        acc[sl] = o["acc_cols"].T.reshape(-1)
        read_w[sl] = o["readw"]
        context[sl] = o["ctxt"].T
        steric_sum += float(o["steric"].sum())
    steric = np.float32(steric_sum / n)
    return sd, acc, steric, read_w, context


# revision 16
# speedup vs baseline: 5.4670x; 5.4670x over previous
"""Trainium2 Bass kernel for NeuralImplicitAccessibilityField.

Data-parallel over N=262144 query points on 8 NeuronCores (32768 queries
per core); residues, anchors and weights replicated.

Outputs (matching reference): (signed_distance[N], accessibility[N],
steric_loss, read_w[N,128], context[N,64]).
"""
import sys
import numpy as np

sys.path.insert(0, "/opt/trn_rl_repo")

import concourse.bass as bass  # noqa: E402
import concourse.bacc as bacc  # noqa: E402
import concourse.tile as tile  # noqa: E402
from concourse import mybir  # noqa: E402

F32 = mybir.dt.float32
BF16 = mybir.dt.bfloat16
AF = mybir.ActivationFunctionType
ALU = mybir.AluOpType
AX = mybir.AxisListType

N_CORES = 8
M = 1024      # residues
A = 128       # anchors
MEM = 64
SUB = 128     # queries per subtile
GRP = 512     # queries per group (4 subtiles)



# ---------------------------------------------------------------------------

def build_module(nq: int, radius: float, sr_b2: float):
    import os
    PH = os.environ.get("K_PHASES", "ABD")
    """Build the per-core Bass module for nq queries (nq % 2048 == 0)."""
    nsub = nq // SUB          # subtiles of 128 queries
    ngrp = nq // GRP          # groups of 512 queries
    npass = (nsub + 127) // 128  # passes of <=128 subtiles
    sub_per_pass = nsub // npass

    nc = bacc.Bacc("TRN2", target_bir_lowering=False, debug=False,
                   num_devices=N_CORES)

    # ---- DRAM parameters -------------------------------------------------
    din = {}

    def dram_in(name, shape):
        din[name] = nc.declare_dram_parameter(name, list(shape), F32,
                                              isOutput=False)
        return din[name]

    c5t = dram_in("c5t", [5, nq])
    r5 = dram_in("r5", [5, M])
    keys2t = dram_in("keys2t", [MEM, 2 * A])
    cpw1 = dram_in("cpw1", [3, MEM])
    cpb1 = dram_in("cpb1", [MEM, 1])
    cpb1h = dram_in("cpb1h", [MEM, 1])
    cpw2h = dram_in("cpw2h", [MEM, MEM])
    cpb2 = dram_in("cpb2", [MEM, 1])
    valh = dram_in("valh", [A, MEM])
    ctrlh = dram_in("ctrlh", [A, MEM])
    slots = dram_in("slots", [A, MEM])
    srw1 = dram_in("srw1", [2 * MEM, MEM])
    wsdfrep = dram_in("wsdfrep", [128, MEM])
    srb1 = dram_in("srb1", [MEM, 1])
    srb1h = dram_in("srb1h", [MEM, 1])
    srw2h = dram_in("srw2h", [MEM, 1])
    ident = dram_in("ident", [128, 128])

    sd_cols_d = nc.declare_dram_parameter("sd_cols", [128, nsub], F32,
                                          isOutput=True)
    acc_cols_d = nc.declare_dram_parameter("acc_cols", [128, nsub], F32,
                                           isOutput=True)
    readw_d = nc.declare_dram_parameter("readw", [nq, A], F32, isOutput=True)
    ctxt_d = nc.declare_dram_parameter("ctxt", [MEM, nq], F32, isOutput=True)
    steric_d = nc.declare_dram_parameter("steric", [128, 1], F32,
                                         isOutput=True)
    resrow_d = nc.dram_tensor("resrow", [nq // GRP, GRP], F32)

    with tile.TileContext(nc) as tc:
        import contextlib
        with contextlib.ExitStack() as ctx:
            singles = ctx.enter_context(tc.tile_pool(name="singles", bufs=1))
            persist = ctx.enter_context(tc.tile_pool(name="persist", bufs=1))

            # ---- constants into SBUF ------------------------------------
            def load_const(ap, shape):
                nm = ap.name + "_s"
                t = singles.tile(list(shape), F32, name=nm, tag=nm)
                nc.sync.dma_start(t[:], ap[:])
                return t

            r5_s = load_const(r5, [5, M])
            keys2t_s = load_const(keys2t, [MEM, 2 * A])
            cpw1_s = load_const(cpw1, [3, MEM])
            cpb1_s = load_const(cpb1, [MEM, 1])
            cpb1h_s = load_const(cpb1h, [MEM, 1])
            cpw2h_s = load_const(cpw2h, [MEM, MEM])
            cpb2_s = load_const(cpb2, [MEM, 1])
            valh_s = load_const(valh, [A, MEM])
            ctrlh_s = load_const(ctrlh, [A, MEM])
            slots_s = load_const(slots, [A, MEM])
            srw1_s = load_const(srw1, [2 * MEM, MEM])
            wsdfrep_s = load_const(wsdfrep, [128, MEM])
            srb1_s = load_const(srb1, [MEM, 1])
            srb1h_s = load_const(srb1h, [MEM, 1])
            srw2h_s = load_const(srw2h, [MEM, 1])
            ident_s = load_const(ident, [128, 128])

            # ---- persistent SBUF tiles ----------------------------------
            qts_all = persist.tile([MEM, nq], F32)        # q^T, feature-major
            min2_cols = persist.tile([128, nsub], F32)    # min d2 per subtile
            sdf_cols = persist.tile([128, nsub], F32)
            sd_cols = persist.tile([128, nsub], F32)
            acc_cols = persist.tile([128, nsub], F32)

            # =============================================================
            # Phase A: distance field (d2 + min) and q-MLP
            # =============================================================
            if "A" in PH:
              with contextlib.ExitStack() as actx:
                pool_a = actx.enter_context(tc.tile_pool(name="pa", bufs=2))
                psum_d2p = actx.enter_context(
                    tc.tile_pool(name="pd2", bufs=2, space="PSUM"))
                psum_mlp = actx.enter_context(
                    tc.tile_pool(name="pmlp", bufs=2, space="PSUM"))

                for blk in range(nq // 2048):     # 2048-query superchunks
                    cchunk = pool_a.tile([5, 2048], F32, tag="cchunk")
                    nc.sync.dma_start(cchunk[:],
                                      c5t[:, blk * 2048:(blk + 1) * 2048])
                    for gi in range(4):           # groups of 512 in chunk
                        g = blk * 4 + gi
                        # --- cp MLP (q^T) over 512 queries ---------------
                        ph1 = psum_mlp.tile([MEM, GRP], F32, tag="ph1")
                        nc.tensor.matmul(
                            ph1[:], cpw1_s[:],
                            cchunk[0:3, gi * GRP:(gi + 1) * GRP])
                        u1 = pool_a.tile([MEM, GRP], F32, tag="u1")
                        nc.scalar.activation(u1[:], ph1[:], AF.Identity,
                                             bias=cpb1_s[:])
                        t1 = pool_a.tile([MEM, GRP], F32, tag="t1")
                        nc.scalar.activation(t1[:], ph1[:], AF.Tanh,
                                             bias=cpb1h_s[:], scale=0.5)
                        m1 = pool_a.tile([MEM, GRP], F32, tag="m1")
                        nc.gpsimd.tensor_tensor(m1[:], u1[:], t1[:],
                                                op=ALU.mult)
                        pqt = psum_mlp.tile([MEM, GRP], F32, tag="pqt")
                        nc.tensor.matmul(pqt[:], cpw2h_s[:], u1[:],
                                         start=True, stop=False)
                        nc.tensor.matmul(pqt[:], cpw2h_s[:], m1[:],
                                         start=False, stop=True)
                        nc.scalar.activation(
                            qts_all[:, g * GRP:(g + 1) * GRP], pqt[:],
                            AF.Identity, bias=cpb2_s[:])

                        # --- d2 + min per subtile ------------------------
                        for j in range(4):
                            t = g * 4 + j
                            pd2 = psum_d2p.tile([128, M], F32, tag="pd2")
                            lhs = cchunk[:, gi * GRP + j * SUB:
                                         gi * GRP + (j + 1) * SUB]
                            nc.tensor.matmul(pd2[:, 0:512], lhs, r5_s[:, 0:512])
                            nc.tensor.matmul(pd2[:, 512:1024], lhs,
                                             r5_s[:, 512:1024])
                            nc.vector.tensor_reduce(
                                min2_cols[:, t:t + 1], pd2[:],
                                axis=AX.X, op=ALU.min)

            # =============================================================
            # Interlude: sdf_base = sqrt(max(min2,0)) - radius; transposes
            # =============================================================
            if "A" in PH:
              with contextlib.ExitStack() as ictx:
                ipool = ictx.enter_context(tc.tile_pool(name="ip", bufs=1))
                ipsum = ictx.enter_context(
                    tc.tile_pool(name="ips", bufs=1, space="PSUM"))
                clamped = ipool.tile([128, nsub], F32)
                nc.vector.tensor_scalar_max(clamped[:], min2_cols[:], 0.0)
                dist = ipool.tile([128, nsub], F32)
                nc.scalar.activation(dist[:], clamped[:], AF.Sqrt)
                nc.vector.tensor_scalar_add(sdf_cols[:], dist[:], -radius)

            # =============================================================
            # Phase B: attention + feature build + final MLP per group
            # =============================================================
            if "B" in PH:
              with contextlib.ExitStack() as bctx:
                pool_b = bctx.enter_context(tc.tile_pool(name="pb", bufs=2))
                ps_log = bctx.enter_context(
                    tc.tile_pool(name="pslog", bufs=1, space="PSUM"))
                ps_tr = bctx.enter_context(
                    tc.tile_pool(name="pstr", bufs=1, space="PSUM"))
                ps_feat = bctx.enter_context(
                    tc.tile_pool(name="psfeat", bufs=2, space="PSUM"))
                ps_ht = bctx.enter_context(
                    tc.tile_pool(name="psht", bufs=1, space="PSUM"))
                ps_res = bctx.enter_context(
                    tc.tile_pool(name="psres", bufs=1, space="PSUM"))

                readw_v = readw_d.ap().rearrange("(t p) c -> p t c", p=SUB)

                for g in range(ngrp):
                    p = (g * 4) // sub_per_pass
                    # --- logits ------------------------------------------
                    plog = ps_log.tile([128, 2 * GRP], F32, tag="plog")
                    for j in range(4):
                        t = g * 4 + j
                        qslice = qts_all[:, t * SUB:(t + 1) * SUB]
                        nc.tensor.matmul(plog[:, j * SUB:(j + 1) * SUB],
                                         qslice, keys2t_s[:, 0:A])
                        nc.tensor.matmul(plog[:, GRP + j * SUB:
                                              GRP + (j + 1) * SUB],
                                         qslice, keys2t_s[:, A:2 * A])
                    expa = pool_b.tile([128, GRP], F32, tag="expa")
                    nc.scalar.activation(expa[:], plog[:, 0:GRP], AF.Exp)
                    expr = pool_b.tile([128, GRP], F32, tag="expr")
                    nc.scalar.activation(expr[:], plog[:, GRP:2 * GRP], AF.Exp)

                    sums8 = pool_b.tile([128, 8], F32, tag="sums8")
                    if os.environ.get("K_REDSUMS"):
                        nc.vector.tensor_reduce(
                            sums8[:, 0:4],
                            expa.rearrange("p (j a) -> p j a", j=4),
                            axis=AX.X, op=ALU.add)
                        nc.vector.tensor_reduce(
                            sums8[:, 4:8],
                            expr.rearrange("p (j a) -> p j a", j=4),
                            axis=AX.X, op=ALU.add)
                    else:
                        tsscr = pool_b.tile([128, GRP], BF16, tag="tsscr")
                        for j in range(4):
                            sl = slice(j * SUB, (j + 1) * SUB)
                            nc.vector.tensor_scalar(
                                out=tsscr[:, sl], in0=expa[:, sl],
                                scalar1=1.0, scalar2=None, op0=ALU.mult,
                                op1=ALU.add,
                                accum_out=sums8[:, j:j + 1])
                            nc.vector.tensor_scalar(
                                out=tsscr[:, sl], in0=expr[:, sl],
                                scalar1=1.0, scalar2=None, op0=ALU.mult,
                                op1=ALU.add,
                                accum_out=sums8[:, 4 + j:5 + j])
                    recip8 = pool_b.tile([128, 8], F32, tag="recip8")
                    if os.environ.get("K_NORECIP"):
                        nc.vector.tensor_copy(recip8[:], sums8[:])
                    else:
                        nc.vector.reciprocal(recip8[:], sums8[:])

                    attn_n = pool_b.tile([128, GRP], F32, tag="attn_n")
                    readw_n = pool_b.tile([128, GRP], F32, tag="readw_n")
                    for j in range(4):
                        sl = slice(j * SUB, (j + 1) * SUB)
                        nc.vector.tensor_scalar_mul(
                            attn_n[:, sl], expa[:, sl], recip8[:, j:j + 1])
                        nc.vector.tensor_scalar_mul(
                            readw_n[:, sl], expr[:, sl], recip8[:, 4 + j:5 + j])
                    nc.sync.dma_start(
                        readw_v[:, g * 4:(g + 1) * 4, :],
                        readw_n.rearrange("p (j a) -> p j a", j=4))

                    if os.environ.get("K_BSTOP") == "1":
                        continue
                    # --- transposes --------------------------------------
                    ptr = ps_tr.tile([128, 2 * GRP], F32, tag="ptr")
                    for j in range(4):
                        sl = slice(j * SUB, (j + 1) * SUB)
                        nc.tensor.transpose(ptr[:, sl], attn_n[:, sl],
                                            ident_s[:])
                        nc.tensor.transpose(
                            ptr[:, GRP + j * SUB:GRP + (j + 1) * SUB],
                            readw_n[:, sl], ident_s[:])
                    ats = pool_b.tile([128, GRP], F32, tag="ats")
                    nc.scalar.copy(ats[:], ptr[:, 0:GRP])
                    rts = pool_b.tile([128, GRP], F32, tag="rts")
                    nc.vector.tensor_copy(rts[:], ptr[:, GRP:2 * GRP])

                    # --- context / memctx (feature-major) ----------------
                    pfeat = ps_feat.tile([128, GRP], F32, tag="pfeat")
                    for j in range(4):
                        sl = slice(j * SUB, (j + 1) * SUB)
                        nc.tensor.matmul(pfeat[0:MEM, sl], valh_s[:],
                                         ats[:, sl], start=True, stop=False)
                        nc.tensor.matmul(pfeat[0:MEM, sl], ctrlh_s[:],
                                         rts[:, sl], start=False, stop=True)
                        nc.tensor.matmul(pfeat[MEM:2 * MEM, sl], slots_s[:],
                                         rts[:, sl])
                    featt = pool_b.tile([128, GRP], F32, tag="featt")
                    nc.scalar.copy(featt[:], pfeat[:])
                    nc.sync.dma_start(ctxt_d[:, g * GRP:(g + 1) * GRP],
                                      featt[0:MEM, :])

                    if os.environ.get("K_BSTOP") == "2":
                        continue
                    # --- final MLP ---------------------------------------
                    pht = ps_ht.tile([MEM, GRP], F32, tag="pht")
                    nc.tensor.matmul(pht[:], srw1_s[:], featt[:],
                                     start=True, stop=False)
                    for j in range(4):
                        t = g * 4 + j
                        diag_t = pool_b.tile([128, 128], F32, tag="diag_t")
                        nc.vector.tensor_scalar_mul(
                            diag_t[:], ident_s[:], sdf_cols[:, t:t + 1])
                        nc.tensor.matmul(
                            pht[:, j * SUB:(j + 1) * SUB], wsdfrep_s[:],
                            diag_t[:], start=False, stop=(j == 3))
                    u2 = pool_b.tile([MEM, GRP], F32, tag="u2")
                    nc.scalar.activation(u2[:], pht[:], AF.Identity,
                                         bias=srb1_s[:])
                    t2 = pool_b.tile([MEM, GRP], F32, tag="t2")
                    nc.scalar.activation(t2[:], pht[:], AF.Tanh,
                                         bias=srb1h_s[:], scale=0.5)
                    m2 = pool_b.tile([MEM, GRP], F32, tag="m2")
                    nc.gpsimd.tensor_tensor(m2[:], u2[:], t2[:], op=ALU.mult)
                    if os.environ.get("K_BSTOP") == "3":
                        continue
                    pres = ps_res.tile([1, GRP], F32, tag="pres")
                    nc.tensor.matmul(pres[:], srw2h_s[:], u2[:],
                                     start=True, stop=False)
                    nc.tensor.matmul(pres[:], srw2h_s[:], m2[:],
                                     start=False, stop=True)
                    if os.environ.get("K_BSTOP") == "4":
                        continue
                    rrow = pool_b.tile([1, GRP], F32, tag="rrow")
                    nc.vector.tensor_copy(rrow[:], pres[:])
                    nc.sync.dma_start(resrow_d[g:g + 1, :], rrow[:])

            # =============================================================
            # Phase D: sd, accessibility, steric, DMAs
            # =============================================================
            if "D" in PH:
              with contextlib.ExitStack() as dctx:
                dpool = dctx.enter_context(tc.tile_pool(name="dp", bufs=1))
                dpsum = dctx.enter_context(
                    tc.tile_pool(name="dps", bufs=1, space="PSUM"))
                rcols_raw = dpool.tile([128, nsub], F32, tag="rcols_raw")
                nc.sync.dma_start(
                    rcols_raw[:],
                    resrow_d.ap().rearrange("g (j p) -> p (g j)", p=SUB))
                rc = dpool.tile([128, nsub], F32, tag="rc")
                nc.scalar.activation(rc[:], rcols_raw[:], AF.Tanh,
                                     bias=float(sr_b2))
                nc.vector.tensor_scalar_mul(rc[:], rc[:], 0.25)
                nc.vector.tensor_add(sd_cols[:], sdf_cols[:], rc[:])
                ta = dpool.tile([128, nsub], F32, tag="ta")
                nc.scalar.activation(ta[:], sd_cols[:], AF.Tanh, scale=4.0)
                nc.vector.tensor_scalar(
                    out=acc_cols[:], in0=ta[:], scalar1=0.5, scalar2=0.5,
                    op0=ALU.mult, op1=ALU.add)
                relu_t = dpool.tile([128, nsub], F32, tag="relu_t")
                steric_t = dpool.tile([128, 1], F32, tag="steric_t")
                nc.scalar.activation(relu_t[:], sd_cols[:], AF.Relu,
                                     scale=-1.0, accum_out=steric_t[:])
                nc.sync.dma_start(sd_cols_d[:], sd_cols[:])
                nc.sync.dma_start(acc_cols_d[:], acc_cols[:])
                nc.sync.dma_start(steric_d[:], steric_t[:])

    if not nc.is_finalized():
        nc.finalize()
    return nc


# ---------------------------------------------------------------------------

def _prep_host(inputs, nq_core):
    """Host-side preprocessing: per-core shards + replicated constants."""
    f32 = np.float32
    coords = np.ascontiguousarray(inputs["coords"], dtype=f32)
    res = np.ascontiguousarray(inputs["residue_positions"], dtype=f32)
    rtypes = np.asarray(inputs["residue_types"]).astype(np.int64)
    radii = np.asarray(inputs["radius_table"], dtype=f32)[rtypes][:, 0]
    uradii = np.unique(radii)
    assert uradii.size == 1, "kernel assumes a single residue radius"
    radius = float(uradii[0])

    n = coords.shape[0]
    c5t = np.empty((5, n), dtype=f32)
    c5t[0:3] = coords.T
    c5t[3] = 1.0
    c5t[4] = (coords.astype(np.float32) ** 2).sum(-1)

    r5 = np.empty((5, M), dtype=f32)
    r5[0:3] = (-2.0 * res).T
    r5[3] = (res ** 2).sum(-1)
    r5[4] = 1.0

    def silu(x):
        return x / (1.0 + np.exp(-x))

    sf = np.asarray(inputs["scalar_features"], dtype=f32)
    aa = np.asarray(inputs["attention_anchors"], dtype=f32)
    mem_in = np.concatenate([sf, aa], -1)
    ctrl = (silu(mem_in @ inputs["mw_w1"] + inputs["mw_b1"])
            @ inputs["mw_w2"] + inputs["mw_b2"]).astype(f32)
    keys = (aa @ inputs["ak_w"] + inputs["ak_b"]).astype(f32)
    values = (mem_in @ inputs["av_w"] + inputs["av_b"]).astype(f32)
    nk = np.asarray(inputs["nftm_keys"], dtype=f32)
    nslots = np.asarray(inputs["nftm_slots"], dtype=f32)

    const = {
        "r5": r5,
        "keys2t": np.concatenate([keys.T, nk.T], axis=1) / 8.0,
        "cpw1": np.asarray(inputs["cp_w1"], dtype=f32),
        "cpb1": np.asarray(inputs["cp_b1"], dtype=f32).reshape(MEM, 1),
        "cpb1h": 0.5 * np.asarray(inputs["cp_b1"], dtype=f32).reshape(MEM, 1),
        "cpw2h": 0.5 * np.asarray(inputs["cp_w2"], dtype=f32),
        "cpb2": np.asarray(inputs["cp_b2"], dtype=f32).reshape(MEM, 1),
        "valh": 0.5 * values,
        "ctrlh": 0.5 * ctrl,
        "slots": nslots,
        "srw1": np.asarray(inputs["sr_w1"], dtype=f32)[0:2 * MEM],
        "wsdfrep": np.tile(
            np.asarray(inputs["sr_w1"], dtype=f32)[2 * MEM:2 * MEM + 1],
            (128, 1)),
        "srb1": np.asarray(inputs["sr_b1"], dtype=f32).reshape(MEM, 1),
        "srb1h": 0.5 * np.asarray(inputs["sr_b1"], dtype=f32).reshape(MEM, 1),
        "srw2h": 0.5 * np.asarray(inputs["sr_w2"], dtype=f32),
        "ident": np.eye(128, dtype=f32),
    }
    const = {k: np.ascontiguousarray(v, dtype=f32) for k, v in const.items()}

    in_maps = []
    for c in range(N_CORES):
        m = dict(const)
        m["c5t"] = np.ascontiguousarray(
            c5t[:, c * nq_core:(c + 1) * nq_core])
        in_maps.append(m)
    sr_b2 = float(np.asarray(inputs["sr_b2"]).reshape(-1)[0])
    return in_maps, radius, sr_b2


def kernel(**inputs):
    from concourse.bass_utils import run_bass_kernel_spmd

    n = inputs["coords"].shape[0]
    assert n % N_CORES == 0
    nq_core = n // N_CORES

    in_maps, radius, sr_b2 = _prep_host(inputs, nq_core)
    nc = build_module(nq_core, radius, sr_b2)
    res = run_bass_kernel_spmd(nc, in_maps, list(range(N_CORES)))

    nsub = nq_core // SUB
    sd = np.empty(n, dtype=np.float32)
    acc = np.empty(n, dtype=np.float32)
    read_w = np.empty((n, A), dtype=np.float32)
    context = np.empty((n, MEM), dtype=np.float32)
    steric_sum = 0.0
    for c in range(N_CORES):
        o = res.results[c]
        sl = slice(c * nq_core, (c + 1) * nq_core)
        sd[sl] = o["sd_cols"].T.reshape(-1)
        acc[sl] = o["acc_cols"].T.reshape(-1)
        read_w[sl] = o["readw"]
        context[sl] = o["ctxt"].T
        steric_sum += float(o["steric"].sum())
    steric = np.float32(steric_sum / n)
    return sd, acc, steric, read_w, context
